# revision 1
# baseline (speedup 1.0000x reference)
"""Bass/Trainium2 kernel for nn_KernelEdges (gnn_message_passing).

Computes A = exp((g_i + g_j - 2*Xf@Xf.T)/sigma^2) with zeroed diagonal,
broadcast to all B batch slots, where Xf = X.transpose(1,0,2).reshape(N, B*d).

Sharding: rows of the NxN pairwise matrix are split across 8 NeuronCores
(256 rows each).  Each core receives the full transposed operand
XT = Xf.T [B*d, N] (host-prepared, 4 MB), its own column-slice as the
stationary matmul operand, and writes its [B, N/8, N] output slice.

Per-core device work:
  psum[mt,nb] = sum_q XT_q[:, m_slice].T @ XT_q[:, n_block]     (Gram matrix)
              + (-1/2*ones).T @ g_row[n_block]                  (rank-1: -g_j/2)
  A = exp(-2/sigma^2 * psum + g_i/sigma^2)                      (ACT, bias per row)
  DMA A tile to the 8 batch slots of the output.

The diagonal is zeroed on the host (16K elements) after the gather.
"""

import numpy as np

B, N, D = 8, 2048, 64
NCORES = 8
R = N // NCORES          # 256 rows per core
KD = B * D               # 512 contraction dim
NB = 512                 # n-block (one PSUM bank of fp32)
NNB = N // NB            # 4 n-blocks
NMT = R // 128           # 2 m-tiles per core
NQ = KD // 128           # 4 k-tiles

# matmul operand dtype: "f32r" (full-rate fp32 mode, ~4e-4 rel err) or
# "bf16" (half the input bytes + faster PE, ~2e-3 rel err)
MM_MODE = "f32r"


def _build_program(inv_s2):
    import concourse.bass as bass
    import concourse.tile as tile
    from concourse import bacc, mybir

    f32 = mybir.dt.float32
    mm_dt = mybir.dt.bfloat16 if MM_MODE == "bf16" else mybir.dt.float32r

    nc = bacc.Bacc(
        "TRN2", target_bir_lowering=False, debug=False, num_devices=NCORES
    )

    GK = 2 if MM_MODE == "bf16" else 1  # g carried as hi+lo rows in bf16

    xt_d = nc.dram_tensor("xt", [KD, N], mm_dt, kind="ExternalInput").ap()
    lhst_d = nc.dram_tensor("lhst", [KD, R], mm_dt, kind="ExternalInput").ap()
    grow_d = nc.dram_tensor("grow", [GK, N], mm_dt, kind="ExternalInput").ap()
    bias_d = nc.dram_tensor("bias", [128, NMT], f32, kind="ExternalInput").ap()
    out_d = nc.dram_tensor("out", [B, R, N], f32, kind="ExternalOutput").ap()

    with tile.TileContext(nc) as tc:
        with (
            tc.tile_pool(name="persist", bufs=1) as persist,
            tc.tile_pool(name="apool", bufs=1) as apool,
            tc.tile_pool(name="psum", bufs=1, space="PSUM") as pspool,
        ):
            # ---- loads ----
            # all input DMAs go on the scalar (ACT) HWDGE ring so the sync
            # ring is dedicated to output DMAs.
            # tiny tensors first: the rank-1 g_j matmuls depend only on
            # these, so they start during the xt load and warm the PE
            grow_sb = persist.tile([GK, N], mm_dt, name="grow")
            nc.scalar.dma_start(grow_sb[:], grow_d[:])

            bias_sb = persist.tile([128, NMT], f32, name="bias")
            nc.scalar.dma_start(bias_sb[:], bias_d[:])

            neg_half = persist.tile([GK, 128], mm_dt, name="neg_half")
            # -0.5 bit pattern; direct float memset into f32r fails ISA check
            if MM_MODE == "bf16":
                nc.gpsimd.memset(
                    neg_half[:].bitcast(mybir.dt.uint16), 0xBF00
                )
            else:
                nc.gpsimd.memset(
                    neg_half[:].bitcast(mybir.dt.uint32), 0xBF000000
                )

            lhs_sb = persist.tile([128, NQ * R], mm_dt, name="lhs")
            nc.scalar.dma_start(
                lhs_sb[:].rearrange("p (q m) -> p q m", q=NQ),
                lhst_d.rearrange("(q p) m -> p q m", p=128),
            )

            # xt tiles; the last one split into n-block pieces so the
            # trailing piece (which gates the final matmul batch) is small
            xt_sb = []
            for q in range(NQ):
                t = persist.tile([128, N], mm_dt, name=f"xt{q}")
                if q < NQ - 1:
                    nc.scalar.dma_start(t[:], xt_d[q * 128:(q + 1) * 128, :])
                else:
                    for nb in range(NNB):
                        sl = slice(nb * NB, (nb + 1) * NB)
                        nc.scalar.dma_start(
                            t[:, sl], xt_d[q * 128:(q + 1) * 128, sl]
                        )
                xt_sb.append(t)

            # ---- compute + store ----
            # all 8 accumulation chains live in the 8 PSUM banks at once;
            # chain order: rank-1 (g_j) first, then k-tiles q0..q3 as each
            # xt_q lands, so the PE overlaps the input DMA
            ps = {}
            for mt in range(NMT):
                for nb in range(NNB):
                    ps[mt, nb] = pspool.tile(
                        [128, NB], f32, name=f"ps{mt}{nb}"
                    )
                    nc.tensor.matmul(
                        ps[mt, nb][:],
                        neg_half[:],
                        grow_sb[:, nb * NB:(nb + 1) * NB],
                        start=True,
                        stop=False,
                    )
            a_sb = {
                mt: apool.tile([128, N], f32, name=f"a{mt}")
                for mt in range(NMT)
            }
            for q in range(NQ):
                last = q == NQ - 1
                # last k-tile arrives in nb pieces: nb-major order so each
                # piece unblocks its matmuls immediately (PE is in-order)
                order = (
                    [(mt, nb) for nb in range(NNB) for mt in range(NMT)]
                    if last
                    else [
                        (mt, nb)
                        for h in range(2)
                        for mt in range(NMT)
                        for nb in range(2 * h, 2 * h + 2)
                    ]
                )
                for mt, nb in order:
                    m0 = q * R + mt * 128
                    nc.tensor.matmul(
                        ps[mt, nb][:],
                        lhs_sb[:, m0:m0 + 128],
                        xt_sb[q][:, nb * NB:(nb + 1) * NB],
                        start=False,
                        stop=last,
                    )
            # ACTs in mt-major order so mt0's output DMA launches as soon
            # as its four n-blocks are done (Scalar executes in FIFO order)
            for mt in range(NMT):
                for nb in range(NNB):
                    nc.scalar.activation(
                        a_sb[mt][:, nb * NB:(nb + 1) * NB],
                        ps[mt, nb][:],
                        mybir.ActivationFunctionType.Exp,
                        bias=bias_sb[:, mt:mt + 1],
                        scale=-2.0 * inv_s2,
                    )
            # one DMA per m-tile replicates [128, 2048] into all 8 batch
            # slots: 8 KB contiguous runs in DRAM
            for mt in range(NMT):
                src = a_sb[mt][:].rearrange(
                    "p (o n) -> p o n", o=1
                ).broadcast_to([128, B, N])
                dst = out_d[
                    :, mt * 128:(mt + 1) * 128, :
                ].rearrange("b p n -> p b n")
                nc.sync.dma_start(dst, src)

    nc.compile()
    return nc


def _prepare(X, log_sigma):
    """Host prep: returns (inv_s2, in_maps) for run_bass_kernel_spmd."""
    X = np.ascontiguousarray(X, dtype=np.float32)
    assert X.shape == (B, N, D), X.shape

    sigma = float(np.exp(np.float32(log_sigma)))
    inv_s2 = 1.0 / (sigma * sigma)

    # XT[b*D+f, n] = X[b, n, f]
    XT = np.ascontiguousarray(X.transpose(0, 2, 1).reshape(KD, N))
    g = np.einsum("kn,kn->n", XT, XT).astype(np.float32)  # [N]
    if MM_MODE == "bf16":
        import ml_dtypes

        XT = np.ascontiguousarray(XT.astype(ml_dtypes.bfloat16))
        g_hi = g.astype(ml_dtypes.bfloat16)
        g_lo = (g - g_hi.astype(np.float32)).astype(ml_dtypes.bfloat16)
        grow_np = np.stack([g_hi, g_lo])  # [2, N]
    else:
        grow_np = g[None, :]

    in_maps = []
    for c in range(NCORES):
        r0 = c * R
        bias_np = np.empty((128, NMT), dtype=np.float32)
        for mt in range(NMT):
            bias_np[:, mt] = g[r0 + mt * 128: r0 + (mt + 1) * 128] * inv_s2
        in_maps.append({
            "xt": XT,
            "lhst": np.ascontiguousarray(XT[:, r0:r0 + R]),
            "grow": grow_np,
            "bias": bias_np,
        })
    return inv_s2, in_maps


def kernel(X, log_sigma):
    from concourse.bass_utils import run_bass_kernel_spmd

    inv_s2, in_maps = _prepare(X, log_sigma)
    nc = _build_program(inv_s2)
    res = run_bass_kernel_spmd(nc, in_maps, list(range(NCORES)))
    out = np.concatenate([res.results[c]["out"] for c in range(NCORES)], axis=1)
    idx = np.arange(N)
    out[:, idx, idx] = 0.0
    return out



# revision 3
# speedup vs baseline: 2.7491x; 2.7491x over previous
"""Bass/Trainium2 kernel for nn_KernelEdges (gnn_message_passing).

Computes A = exp((g_i + g_j - 2*Xf@Xf.T)/sigma^2) with zeroed diagonal,
broadcast to all B batch slots, where Xf = X.transpose(1,0,2).reshape(N, B*d).

Sharding: rows of the NxN pairwise matrix are split across 8 NeuronCores
(256 rows each).  A is identical in every batch slot, so each core writes
its [N/8, N] row tile exactly once (fp16); the host broadcasts over B.

Per-core inputs are column-ROTATED by the core's row offset so the SPMD
program can take its stationary (LHS) matmul operand from a fixed slice
xt[:, 0:256] of the replicated matrix: core c receives
xt_c[:, j] = XT_bf16[:, (j + c*R) % N].  The host un-rotates the output
columns with np.roll after the gather.

The g_j (per-column) term is applied multiplicatively: the device computes
E = exp(g_i/s^2 - 2*dot/s^2) on the scalar engine (per-row bias) and the
otherwise-idle vector engine multiplies by v_j = exp(g_j/s^2), which the
host sends pre-replicated across the 128 partitions.  This keeps the
tensor engine at exactly the 32 Gram-tile matmuls (no rank-1 chains).

Per-core device work (bf16 matmul, fp32 PSUM accumulation):
  psum[mt,nb] = sum_q xt_q[:, mt].T @ xt_q[:, nb]      (Gram tile)
  e = exp(-2/sigma^2 * psum + g_i/sigma^2)             (ACT, fp16)
  a = e * v_j                                          (DVE, fp16)
  DMA a[mt, half] to out[R, N] once.

The diagonal is zeroed on the host (2048 elements) after the gather.
"""

import numpy as np

B, N, D = 8, 2048, 64
NCORES = 8
R = N // NCORES          # 256 rows per core
KD = B * D               # 512 contraction dim
NB = 512                 # n-block (one PSUM bank of fp32)
NNB = N // NB            # 4 n-blocks
NH = 2                   # column halves (input/store granularity, 1024 wide)
HW = N // NH             # 1024
NMT = R // 128           # 2 m-tiles per core
NQ = KD // 128           # 4 k-tiles


def _build_program(inv_s2):
    import concourse.bass as bass
    import concourse.tile as tile
    from concourse import bacc, mybir

    f32 = mybir.dt.float32
    f16 = mybir.dt.float16
    bf16 = mybir.dt.bfloat16

    nc = bacc.Bacc(
        "TRN2", target_bir_lowering=False, debug=False, num_devices=NCORES
    )

    xt_d = nc.dram_tensor("xt", [KD, N], bf16, kind="ExternalInput").ap()
    vbb_d = nc.dram_tensor("vbb", [128, N], f16, kind="ExternalInput").ap()
    bias_d = nc.dram_tensor("bias", [128, NMT], f32, kind="ExternalInput").ap()
    out_d = nc.dram_tensor("out", [R, N], f16, kind="ExternalOutput").ap()

    with tile.TileContext(nc) as tc:
        with (
            tc.tile_pool(name="persist", bufs=1) as persist,
            tc.tile_pool(name="apool", bufs=1) as apool,
            tc.tile_pool(name="psum", bufs=1, space="PSUM") as pspool,
        ):
            xt_sb = [
                persist.tile([128, N], bf16, name=f"xt{q}") for q in range(NQ)
            ]
            vbb_sb = persist.tile([128, N], f16, name="vbb")
            bias_sb = persist.tile([128, NMT], f32, name="bias")

            # ---- input streaming ----
            # scalar ring: first column-half of xt (the piece the PE needs
            # first), then the tiny bias; the 8 exp ACTs queue after.
            for q in range(NQ):
                nc.scalar.dma_start(
                    xt_sb[q][:, 0:HW], xt_d[q * 128:(q + 1) * 128, 0:HW]
                )
            nc.scalar.dma_start(bias_sb[:], bias_d[:])
            # sync ring: v (needed from the first DVE multiply on), the
            # second column-half, then the output stores queue behind.
            nc.sync.dma_start(vbb_sb[:], vbb_d[:])
            for q in range(NQ):
                nc.sync.dma_start(
                    xt_sb[q][:, HW:N], xt_d[q * 128:(q + 1) * 128, HW:N]
                )

            # ---- compute ----
            # 8 accumulation chains in the 8 PSUM banks, half-major so the
            # first half's ACT/mult/store overlaps the second half's load.
            ps = {
                (mt, nb): pspool.tile([128, NB], f32, name=f"ps{mt}{nb}")
                for mt in range(NMT)
                for nb in range(NNB)
            }
            e_sb = {
                mt: apool.tile([128, N], f16, name=f"e{mt}")
                for mt in range(NMT)
            }
            a_sb = {
                mt: apool.tile([128, N], f16, name=f"a{mt}")
                for mt in range(NMT)
            }
            for h in range(NH):
                nbs = (2 * h, 2 * h + 1)
                for q in range(NQ):
                    for mt in range(NMT):
                        for nb in nbs:
                            nc.tensor.matmul(
                                ps[mt, nb][:],
                                xt_sb[q][:, mt * 128:(mt + 1) * 128],
                                xt_sb[q][:, nb * NB:(nb + 1) * NB],
                                start=q == 0,
                                stop=q == NQ - 1,
                            )
                for mt in range(NMT):
                    for nb in nbs:
                        sl = slice(nb * NB, (nb + 1) * NB)
                        nc.scalar.activation(
                            e_sb[mt][:, sl],
                            ps[mt, nb][:],
                            mybir.ActivationFunctionType.Exp,
                            bias=bias_sb[:, mt:mt + 1],
                            scale=-2.0 * inv_s2,
                        )
                        nc.vector.scalar_tensor_tensor(
                            a_sb[mt][:, sl],
                            e_sb[mt][:, sl],
                            1.0,
                            vbb_sb[:, sl],
                            mybir.AluOpType.bypass,
                            mybir.AluOpType.mult,
                        )
                for mt in range(NMT):
                    sl = slice(h * HW, (h + 1) * HW)
                    nc.sync.dma_start(
                        out_d[mt * 128:(mt + 1) * 128, sl],
                        a_sb[mt][:, sl],
                    )

    nc.compile()
    return nc


def _prepare(X, log_sigma):
    """Host prep: returns (inv_s2, in_maps) for run_bass_kernel_spmd."""
    import ml_dtypes

    X = np.ascontiguousarray(X, dtype=np.float32)
    assert X.shape == (B, N, D), X.shape

    sigma = float(np.exp(np.float32(log_sigma)))
    inv_s2 = 1.0 / (sigma * sigma)

    # XT[b*D+f, n] = X[b, n, f]
    XT = np.ascontiguousarray(X.transpose(0, 2, 1).reshape(KD, N))
    g = np.einsum("kn,kn->n", XT, XT).astype(np.float32)  # [N]

    XTb = XT.astype(ml_dtypes.bfloat16)
    v16 = np.exp(g * inv_s2).astype(np.float16)  # [N]

    in_maps = []
    for c in range(NCORES):
        r0 = c * R
        bias_np = np.empty((128, NMT), dtype=np.float32)
        for mt in range(NMT):
            bias_np[:, mt] = g[r0 + mt * 128: r0 + (mt + 1) * 128] * inv_s2
        vr = np.roll(v16, -r0)
        in_maps.append({
            "xt": np.ascontiguousarray(np.roll(XTb, -r0, axis=1)),
            "vbb": np.ascontiguousarray(
                np.broadcast_to(vr[None, :], (128, N))
            ),
            "bias": bias_np,
        })
    return inv_s2, in_maps


def kernel(X, log_sigma):
    from concourse.bass_utils import run_bass_kernel_spmd

    inv_s2, in_maps = _prepare(X, log_sigma)
    nc = _build_program(inv_s2)
    res = run_bass_kernel_spmd(nc, in_maps, list(range(NCORES)))
    A16 = np.empty((N, N), dtype=np.float16)
    for c in range(NCORES):
        r0 = c * R
        A16[r0:r0 + R, :] = np.roll(res.results[c]["out"], r0, axis=1)
    A = A16.astype(np.float32)
    idx = np.arange(N)
    A[idx, idx] = 0.0
    out = np.empty((B, N, N), dtype=np.float32)
    out[:] = A[None, :, :]
    return out


# revision 4
# speedup vs baseline: 2.7798x; 1.0112x over previous
"""Bass/Trainium2 kernel for nn_KernelEdges (gnn_message_passing).

Computes A = exp((g_i + g_j - 2*Xf@Xf.T)/sigma^2) with zeroed diagonal,
broadcast to all B batch slots, where Xf = X.transpose(1,0,2).reshape(N, B*d).

Sharding: rows of the NxN pairwise matrix are split across 8 NeuronCores
(256 rows each).  A is identical in every batch slot, so each core writes
its [N/8, N] row tile exactly once (fp16); the host broadcasts over B.

Per-core inputs are column-ROTATED by the core's row offset so the SPMD
program can take its stationary (LHS) matmul operand from a fixed slice
xt[:, 0:256] of the replicated matrix: core c receives
xt_c[:, j] = XT_bf16[:, (j + c*R) % N].  The host un-rotates the output
columns with np.roll after the gather.

The g_j (per-column) term is applied multiplicatively: the device computes
E = exp(g_i/s^2 - 2*dot/s^2) on the scalar engine (per-row bias) and the
otherwise-idle vector engine multiplies by v_j = exp(g_j/s^2) (tensor_mul,
which runs in the DVE 2x fp16 mode).  v arrives as a [1, N] row and is
replicated across the 128 partitions by a stride-0 broadcast DMA.

The tensor engine starts in a low DVFS p-state (half speed for the first
~3us of busy time), so a handful of warm-up matmuls on scratch data run
during the input load to get the ramp out of the way.

Per-core device work (bf16 matmul, fp32 PSUM accumulation):
  psum[mt,nb] = sum_q xt_q[:, mt].T @ xt_q[:, nb]      (Gram tile)
  e = exp(-2/sigma^2 * psum + g_i/sigma^2)             (ACT, fp16)
  a = e * v_j                                          (DVE, fp16)
  DMA a[mt, half] to out[R, N] once.

The diagonal is zeroed on the host (2048 elements) after the gather.
"""

import numpy as np

B, N, D = 8, 2048, 64
NCORES = 8
R = N // NCORES          # 256 rows per core
KD = B * D               # 512 contraction dim
NB = 512                 # n-block (one PSUM bank of fp32)
NNB = N // NB            # 4 n-blocks
NH = 2                   # column halves (input/store granularity, 1024 wide)
HW = N // NH             # 1024
NMT = R // 128           # 2 m-tiles per core
NQ = KD // 128           # 4 k-tiles
NWARM = 6                # PE p-state warm-up matmuls


def _build_program(inv_s2):
    import concourse.bass as bass
    import concourse.tile as tile
    from concourse import bacc, mybir

    f32 = mybir.dt.float32
    f16 = mybir.dt.float16
    bf16 = mybir.dt.bfloat16

    nc = bacc.Bacc(
        "TRN2", target_bir_lowering=False, debug=False, num_devices=NCORES
    )

    xt_d = nc.dram_tensor("xt", [KD, N], bf16, kind="ExternalInput").ap()
    v_d = nc.dram_tensor("v", [1, N], f16, kind="ExternalInput").ap()
    bias_d = nc.dram_tensor("bias", [128, NMT], f32, kind="ExternalInput").ap()
    out_d = nc.dram_tensor("out", [R, N], f16, kind="ExternalOutput").ap()

    with tile.TileContext(nc) as tc:
        with (
            tc.tile_pool(name="persist", bufs=1) as persist,
            tc.tile_pool(name="apool", bufs=1) as apool,
            tc.tile_pool(name="psum", bufs=1, space="PSUM") as pspool,
        ):
            xt_sb = [
                persist.tile([128, N], bf16, name=f"xt{q}") for q in range(NQ)
            ]
            vbb_sb = persist.tile([128, N], f16, name="vbb")
            bias_sb = persist.tile([128, NMT], f32, name="bias")
            wsrc = persist.tile([128, 128 + NB], bf16, name="wsrc")

            ps = {
                (mt, nb): pspool.tile([128, NB], f32, name=f"ps{mt}{nb}")
                for mt in range(NMT)
                for nb in range(NNB)
            }

            # scratch for PE warm-up (gpsimd memset to 1.0 bf16)
            nc.gpsimd.memset(wsrc[:].bitcast(mybir.dt.uint16), 0x3F80)
            # warm-up matmuls: start the tensor engine's DVFS ramp while
            # the inputs stream in; results land in ps[0,0] and are
            # overwritten by that chain's real start=True matmul.
            for _ in range(NWARM):
                nc.tensor.matmul(
                    ps[0, 0][:],
                    wsrc[:, 0:128],
                    wsrc[:, 128:128 + NB],
                    start=True,
                    stop=True,
                )

            # sync ring: tiny tensors + v broadcast (stride-0 DRAM src),
            # stores queue behind.
            nc.sync.dma_start(bias_sb[:], bias_d[:])
            nc.sync.dma_start(vbb_sb[:], v_d.broadcast_to([128, N]))

            # scalar ring: all of xt, half-major so the first half's
            # chains close while the second half loads.
            for h in range(NH):
                sl = slice(h * HW, (h + 1) * HW)
                for q in range(NQ):
                    nc.scalar.dma_start(
                        xt_sb[q][:, sl], xt_d[q * 128:(q + 1) * 128, sl]
                    )

            e_sb = {
                mt: apool.tile([128, N], f16, name=f"e{mt}")
                for mt in range(NMT)
            }
            a_sb = {
                mt: apool.tile([128, N], f16, name=f"a{mt}")
                for mt in range(NMT)
            }
            for h in range(NH):
                nbs = (2 * h, 2 * h + 1)
                for q in range(NQ):
                    for mt in range(NMT):
                        for nb in nbs:
                            nc.tensor.matmul(
                                ps[mt, nb][:],
                                xt_sb[q][:, mt * 128:(mt + 1) * 128],
                                xt_sb[q][:, nb * NB:(nb + 1) * NB],
                                start=q == 0,
                                stop=q == NQ - 1,
                            )
                for mt in range(NMT):
                    for nb in nbs:
                        sl = slice(nb * NB, (nb + 1) * NB)
                        nc.scalar.activation(
                            e_sb[mt][:, sl],
                            ps[mt, nb][:],
                            mybir.ActivationFunctionType.Exp,
                            bias=bias_sb[:, mt:mt + 1],
                            scale=-2.0 * inv_s2,
                        )
                        nc.vector.tensor_mul(
                            a_sb[mt][:, sl],
                            e_sb[mt][:, sl],
                            vbb_sb[:, sl],
                        )
                    hsl = slice(h * HW, (h + 1) * HW)
                    nc.sync.dma_start(
                        out_d[mt * 128:(mt + 1) * 128, hsl],
                        a_sb[mt][:, hsl],
                    )

    nc.compile()
    return nc


def _prepare(X, log_sigma):
    """Host prep: returns (inv_s2, in_maps) for run_bass_kernel_spmd."""
    import ml_dtypes

    X = np.ascontiguousarray(X, dtype=np.float32)
    assert X.shape == (B, N, D), X.shape

    sigma = float(np.exp(np.float32(log_sigma)))
    inv_s2 = 1.0 / (sigma * sigma)

    # XT[b*D+f, n] = X[b, n, f]
    XT = np.ascontiguousarray(X.transpose(0, 2, 1).reshape(KD, N))
    g = np.einsum("kn,kn->n", XT, XT).astype(np.float32)  # [N]

    XTb = XT.astype(ml_dtypes.bfloat16)
    v16 = np.exp(g * inv_s2).astype(np.float16)  # [N]

    in_maps = []
    for c in range(NCORES):
        r0 = c * R
        bias_np = np.empty((128, NMT), dtype=np.float32)
        for mt in range(NMT):
            bias_np[:, mt] = g[r0 + mt * 128: r0 + (mt + 1) * 128] * inv_s2
        in_maps.append({
            "xt": np.ascontiguousarray(np.roll(XTb, -r0, axis=1)),
            "v": np.ascontiguousarray(np.roll(v16, -r0)[None, :]),
            "bias": bias_np,
        })
    return inv_s2, in_maps


def kernel(X, log_sigma):
    from concourse.bass_utils import run_bass_kernel_spmd

    inv_s2, in_maps = _prepare(X, log_sigma)
    nc = _build_program(inv_s2)
    res = run_bass_kernel_spmd(nc, in_maps, list(range(NCORES)))
    A16 = np.empty((N, N), dtype=np.float16)
    for c in range(NCORES):
        r0 = c * R
        A16[r0:r0 + R, :] = np.roll(res.results[c]["out"], r0, axis=1)
    A = A16.astype(np.float32)
    idx = np.arange(N)
    A[idx, idx] = 0.0
    out = np.empty((B, N, N), dtype=np.float32)
    out[:] = A[None, :, :]
    return out


# revision 6
# speedup vs baseline: 3.3311x; 1.1983x over previous
"""Bass/Trainium2 kernel for nn_KernelEdges (gnn_message_passing).

Computes A = exp((g_i + g_j - 2*Xf@Xf.T)/sigma^2) with zeroed diagonal,
broadcast to all B batch slots, where Xf = X.transpose(1,0,2).reshape(N, B*d).

Sharding: rows of the NxN pairwise matrix are split across 8 NeuronCores
(256 rows each).  A is identical in every batch slot, so each core writes
its [N/8, N] row tile exactly once (fp16); the host broadcasts over B.

Per-core inputs are column-ROTATED by the core's row offset so the SPMD
program can take its stationary (LHS) matmul operand from a fixed slice
xt[:, :, 0:256] of the replicated matrix.  The host un-rotates the output
columns with np.roll after the gather.

The Gram matmuls run in fp8(e4m3) DoubleRow mode: the 512-long contraction
is packed as two [128, 2, N] tiles (k-tile pairs in the second dim), so
each chain is 2 matmuls at 0.5 cycles/column.  g is computed in fp32 on
the host, so only the cross-term dot suffers fp8 rounding (absmax rel err
~1.1e-2, vs the 2e-2 gate).

The g_j (per-column) term is applied multiplicatively: the device computes
E = exp(g_i/s^2 - 2*dot/s^2) on the scalar engine (per-row bias) and the
vector engine multiplies by v_j = exp(g_j/s^2) (tensor_mul, DVE 2x fp16
mode).  v arrives as a [1, N] fp16 row and is replicated across the 128
partitions by an SBUF-to-SBUF stride-0 DMA (no HBM traffic).

The tensor engine starts in a low DVFS p-state (half speed for the first
~3us of busy time), so warm-up matmuls on scratch data run during the
input load to get the ramp out of the way.

The diagonal is zeroed on the host (2048 elements) after the gather.
"""

import numpy as np

B, N, D = 8, 2048, 64
NCORES = 8
R = N // NCORES          # 256 rows per core
KD = B * D               # 512 contraction dim
NB = 512                 # n-block (one PSUM bank of fp32)
NH = 2                   # column halves (piece/ACT/store granularity)
HW = N // NH             # 1024
NMT = R // 128           # 2 m-tiles per core
NQP = 2                  # k-tile pairs (DoubleRow: 2x128 contraction each)
NWARM = 6                # PE p-state warm-up matmuls


def _build_program(inv_s2):
    import concourse.bass as bass
    import concourse.tile as tile
    from concourse import bacc, mybir

    f32 = mybir.dt.float32
    f16 = mybir.dt.float16
    fp8 = mybir.dt.float8e4

    nc = bacc.Bacc(
        "TRN2", target_bir_lowering=False, debug=False, num_devices=NCORES
    )

    xt_d = [
        nc.dram_tensor(f"xt{qp}", [128, NQP, N], fp8, kind="ExternalInput").ap()
        for qp in range(NQP)
    ]
    v_d = nc.dram_tensor("v", [1, N], f16, kind="ExternalInput").ap()
    bias_d = nc.dram_tensor("bias", [128, NMT], f32, kind="ExternalInput").ap()
    out_d = nc.dram_tensor("out", [R, N], f16, kind="ExternalOutput").ap()

    with tile.TileContext(nc) as tc:
        with (
            tc.tile_pool(name="persist", bufs=1) as persist,
            tc.tile_pool(name="apool", bufs=1) as apool,
            tc.tile_pool(name="psum", bufs=1, space="PSUM") as pspool,
        ):
            xt_sb = [
                persist.tile([128, NQP, N], fp8, name=f"xt{qp}")
                for qp in range(NQP)
            ]
            v_sb = persist.tile([1, N], f16, name="v")
            vbb_sb = persist.tile([128, N], f16, name="vbb")
            bias_sb = persist.tile([128, NMT], f32, name="bias")
            ones_sb = persist.tile([1, 128], f16, name="ones")
            wsrc = persist.tile([128, 128 + NB], mybir.dt.bfloat16, name="wsrc")

            # [128, 1024] PSUM tiles: two 512-wide accumulation chains each,
            # read back by a single wide ACT.
            ps = {
                (mt, h): pspool.tile([128, HW], f32, name=f"ps{mt}{h}")
                for mt in range(NMT)
                for h in range(NH)
            }

            # PE p-state warm-up on scratch data (results overwritten by the
            # first start=True matmul of the real chains).
            nc.gpsimd.memset(wsrc[:].bitcast(mybir.dt.uint16), 0x3F80)
            nc.gpsimd.memset(ones_sb[:].bitcast(mybir.dt.uint16), 0x3C00)
            for _ in range(NWARM):
                nc.tensor.matmul(
                    ps[0, 0][:, 0:NB],
                    wsrc[:, 0:128],
                    wsrc[:, 128:128 + NB],
                    start=True,
                    stop=True,
                )

            # sync ring: tiny loads; output stores queue behind.
            nc.sync.dma_start(v_sb[:], v_d[:])
            nc.sync.dma_start(bias_sb[:], bias_d[:])

            # Replicate v across partitions on-chip: rank-1 ones.T @ v into
            # the h1 PSUM tiles (consumed last, so no WAR stall), DVE-copied
            # out to SBUF fp16.  Doubles as extra PE warm-up.
            for k in range(4):
                tile_ps = ps[k // 2, 1]
                hb = k % 2
                nc.tensor.matmul(
                    tile_ps[:, hb * NB:(hb + 1) * NB],
                    ones_sb[:],
                    v_sb[0:1, k * NB:(k + 1) * NB],
                    start=True,
                    stop=True,
                )
                nc.vector.tensor_copy(
                    vbb_sb[:, k * NB:(k + 1) * NB],
                    tile_ps[:, hb * NB:(hb + 1) * NB],
                )

            # scalar ring: xt in (half, qp) pieces; the scalar engine then
            # runs the exp ACTs.
            for h in range(NH):
                sl = slice(h * HW, (h + 1) * HW)
                for qp in range(NQP):
                    nc.scalar.dma_start(
                        xt_sb[qp][:, :, sl], xt_d[qp][:, :, sl]
                    )

            e_sb = {
                mt: apool.tile([128, N], f16, name=f"e{mt}")
                for mt in range(NMT)
            }
            a_sb = {
                mt: apool.tile([128, N], f16, name=f"a{mt}")
                for mt in range(NMT)
            }
            for h in range(NH):
                for qp in range(NQP):
                    for mt in range(NMT):
                        for hb in range(2):
                            c0 = h * HW + hb * NB
                            nc.tensor.matmul(
                                ps[mt, h][:, hb * NB:(hb + 1) * NB],
                                xt_sb[qp][:, :, mt * 128:(mt + 1) * 128],
                                xt_sb[qp][:, :, c0:c0 + NB],
                                start=qp == 0,
                                stop=qp == NQP - 1,
                                perf_mode=mybir.MatmulPerfMode.DoubleRow,
                            )
                hsl = slice(h * HW, (h + 1) * HW)
                for mt in range(NMT):
                    nc.scalar.activation(
                        e_sb[mt][:, hsl],
                        ps[mt, h][:],
                        mybir.ActivationFunctionType.Exp,
                        bias=bias_sb[:, mt:mt + 1],
                        scale=-2.0 * inv_s2,
                    )
                    nc.vector.tensor_mul(
                        a_sb[mt][:, hsl],
                        e_sb[mt][:, hsl],
                        vbb_sb[:, hsl],
                    )
                    nc.sync.dma_start(
                        out_d[mt * 128:(mt + 1) * 128, hsl],
                        a_sb[mt][:, hsl],
                    )

    nc.compile()
    return nc


def _prepare(X, log_sigma):
    """Host prep: returns (inv_s2, in_maps) for run_bass_kernel_spmd."""
    import ml_dtypes

    X = np.ascontiguousarray(X, dtype=np.float32)
    assert X.shape == (B, N, D), X.shape

    sigma = float(np.exp(np.float32(log_sigma)))
    inv_s2 = 1.0 / (sigma * sigma)

    # XT[b*D+f, n] = X[b, n, f]
    XT = np.ascontiguousarray(X.transpose(0, 2, 1).reshape(KD, N))
    g = np.einsum("kn,kn->n", XT, XT).astype(np.float32)  # [N]

    XT8 = XT.astype(ml_dtypes.float8_e4m3fn)
    v16 = np.exp(g * inv_s2).astype(np.float16)  # [N]

    in_maps = []
    for c in range(NCORES):
        r0 = c * R
        bias_np = np.empty((128, NMT), dtype=np.float32)
        for mt in range(NMT):
            bias_np[:, mt] = g[r0 + mt * 128: r0 + (mt + 1) * 128] * inv_s2
        # [qp, p, s, n] with k = (2*qp + s)*128 + p
        xtr = np.roll(XT8, -r0, axis=1).reshape(NQP, 2, 128, N)
        xtr = np.ascontiguousarray(xtr.transpose(0, 2, 1, 3))
        im = {
            f"xt{qp}": xtr[qp] for qp in range(NQP)
        }
        im["v"] = np.ascontiguousarray(np.roll(v16, -r0)[None, :])
        im["bias"] = bias_np
        in_maps.append(im)
    return inv_s2, in_maps


def kernel(X, log_sigma):
    from concourse.bass_utils import run_bass_kernel_spmd

    inv_s2, in_maps = _prepare(X, log_sigma)
    nc = _build_program(inv_s2)
    res = run_bass_kernel_spmd(nc, in_maps, list(range(NCORES)))
    A16 = np.empty((N, N), dtype=np.float16)
    for c in range(NCORES):
        r0 = c * R
        A16[r0:r0 + R, :] = np.roll(res.results[c]["out"], r0, axis=1)
    A = A16.astype(np.float32)
    idx = np.arange(N)
    A[idx, idx] = 0.0
    out = np.empty((B, N, N), dtype=np.float32)
    out[:] = A[None, :, :]
    return out
